# revision 44
# baseline (speedup 1.0000x reference)
"""2-layer GCN encoder on 8 TRN2 NeuronCores (Bass/Tile SPMD).

Strategy (per sharding hint): dst-node sharding, 6250 nodes/core.
- Host: compute degrees/norm, build per-core edge streams grouped by
  (dst block of 125 nodes, src parity), padded to 128-edge tiles with
  tile counts uniform across cores (one SPMD program).
- Layer 1: host-gathered, n_e-scaled bf16 messages are streamed on BOTH
  HWDGE queues (h=0 via SP, h=1 via Activation -- doubles DMA engine
  concurrency, 135 -> 115 us; fp8 messages were tried and pushed
  max-rel err to 0.033 > the 0.02 gate), segment-summed per dst block
  via one-hot matmul in PSUM, then W1 -> relu -> W2 fused tail; rows
  scaled by d^-1/2 and written to the bf16 inter-layer table shard.
- AllGather of the 6250x64 bf16 shards; the layer-2 gather index
  streams (~6 MB) are loaded concurrently with the collective (DMA is
  otherwise idle there).  A local/remote edge split that overlapped
  local-source aggregation with the collective was tried and REGRESSED
  (all phases here are HBM-bandwidth-bound, so overlapping two
  DMA-heavy phases just splits bandwidth and stretched the collective
  2.5x; it also added 13% more gather slots).  Deeper tile-pool
  buffering (bufs=3) and feature-major output writes were also tried
  and measured net-negative.
- Layer 2: dma_gather row-pairs from the AllGather'd table (int16 pair
  indices stay < 32768), one-hot scatter into PSUM, scale by
  d^-1/2[dst] (DVE), add b2, transpose, write output rows.
  Gather calls round-robin the 4 SWDGE queues.  single_packet=True
  calls are limited to GSUB=8 tiles (the per-engine packet is
  GSUB*128/16 * 256B and must stay <= PACKET_BYTES=16384; bigger calls
  hang the DMA engines).
"""
import numpy as np
import ml_dtypes

from concourse import bass, bacc, mybir, tile
from concourse.bass_utils import run_bass_kernel_spmd

N_CORES = 8
N = 50000
IN = 128
HID = 128
OUT = 64
NPC = N // N_CORES      # 6250 nodes per core
BW = 125                # dst block width
NB = NPC // BW          # 50 blocks per core
CHB = 5                 # blocks per processing chunk
N_QUEUES = 4            # SWDGE queues to round-robin dma_gather across
DMA_SCRATCH = 32768

# single_packet=False was A/B-tested and lost (fast 256B-packet drains
# but ~2x descriptor-generation cost starves them); keep True + GSUB=8.
GSUB = 8

BF = mybir.dt.bfloat16
F32 = mybir.dt.float32
bf16 = ml_dtypes.bfloat16


def _wrap_idx(idx):
    """dma_gather int16 index layout: [128, n/16]; index i at [i%16, i//16],
    replicated across the 8 gpsimd cores (16-partition groups)."""
    n = len(idx)
    assert n % 128 == 0
    base = np.asarray(idx, dtype=np.int16).reshape(n // 16, 16).T  # [16, n/16]
    return np.tile(base, (8, 1))


def _preprocess(x, edge_index, W1, b1, W2, b2):
    src = np.asarray(edge_index[0], dtype=np.int64)
    dst = np.asarray(edge_index[1], dtype=np.int64)
    loop = np.arange(N, dtype=np.int64)
    src = np.concatenate([src, loop])
    dst = np.concatenate([dst, loop])

    deg = np.bincount(dst, minlength=N).astype(np.float32)
    dinv = (1.0 / np.sqrt(deg)).astype(np.float32)  # deg >= 1 (self loops)

    x32 = np.asarray(x, dtype=np.float32)

    # the appended self loops take part in layer 1 (streamed messages are
    # free to include them) but are EXCLUDED from the layer-2 gather: their
    # contribution dinv^2 * (relu(h1)@W2) is computed densely in the
    # layer-1 tail instead, cutting ~7% of the random fetches and tiles
    is_loop = np.concatenate(
        [np.zeros(len(src) - N, dtype=bool), np.ones(N, dtype=bool)])

    core = dst // NPC
    per_core = []
    cnts = np.zeros((N_CORES, NB, 2), dtype=np.int64)
    cnts2 = np.zeros((N_CORES, NB, 2), dtype=np.int64)
    for m in range(N_CORES):
        sel = core == m
        s = src[sel]
        d = dst[sel] - m * NPC
        b = d // BW
        h = (s % 2).astype(np.int64)
        ne = dinv[s] * dinv[dst[sel]]
        lp = is_loop[sel]
        per_core.append((s, b, h, ne, d % BW, lp))
        cnts[m] = np.bincount(b * 2 + h, minlength=2 * NB).reshape(NB, 2)
        cnts2[m] = np.bincount((b * 2 + h)[~lp],
                               minlength=2 * NB).reshape(NB, 2)

    Tt = np.maximum(1, -(-cnts.max(axis=0) // 128))   # layer-1 tiles
    Tt2 = np.maximum(1, -(-cnts2.max(axis=0) // 128))  # layer-2 tiles

    inputs = []
    for m in range(N_CORES):
        s, b, h, ne, l, lp = per_core[m]
        per_in = {}
        for hh in (0, 1):
            mh = h == hh
            bh, lh, sh, neh = b[mh], l[mh], s[mh], ne[mh]
            # sort by src within each block bucket: ascending gather
            # addresses improve HBM locality
            order = np.lexsort((sh, bh))
            bh, lh, sh, neh = bh[order], lh[order], sh[order], neh[order]
            bounds = np.searchsorted(bh, np.arange(NB + 1))
            chunks_dstl, chunks_msg = [], []
            for bb in range(NB):
                lo, hi = bounds[bb], bounds[bb + 1]
                npad = Tt[bb, hh] * 128 - (hi - lo)
                assert npad >= 0
                chunks_dstl.append(lh[lo:hi])
                chunks_dstl.append(np.full(npad, 126, dtype=np.int64))
                chunks_msg.append(x32[sh[lo:hi]] * neh[lo:hi, None])
                chunks_msg.append(np.zeros((npad, IN), dtype=np.float32))
            dstl = np.concatenate(chunks_dstl)
            ntile = len(dstl) // 128
            per_in[f"dstl{hh}"] = dstl.reshape(ntile, 128).T.astype(bf16).copy()
            msg = np.concatenate(chunks_msg).astype(bf16)
            per_in[f"msg{hh}"] = np.ascontiguousarray(
                msg.reshape(ntile, 128, IN).transpose(1, 0, 2))
            # layer-2 streams: non-loop edges only
            mh2 = mh & ~lp
            bh, lh, sh = b[mh2], l[mh2], s[mh2]
            order = np.lexsort((sh, bh))
            bh, lh, sh = bh[order], lh[order], sh[order]
            bounds = np.searchsorted(bh, np.arange(NB + 1))
            chunks_idx, chunks_dstl = [], []
            for bb in range(NB):
                lo, hi = bounds[bb], bounds[bb + 1]
                npad = Tt2[bb, hh] * 128 - (hi - lo)
                assert npad >= 0
                chunks_idx.append(sh[lo:hi] // 2)
                chunks_idx.append(np.zeros(npad, dtype=np.int64))
                chunks_dstl.append(lh[lo:hi])
                chunks_dstl.append(np.full(npad, 126, dtype=np.int64))
            dstl = np.concatenate(chunks_dstl)
            ntile = len(dstl) // 128
            per_in[f"idx{hh}"] = _wrap_idx(np.concatenate(chunks_idx))
            per_in[f"dstl2_{hh}"] = dstl.reshape(ntile, 128).T.astype(
                bf16).copy()

        mloc = m * NPC
        dinv_loc = dinv[mloc:mloc + NPC]
        per_in["W1"] = np.asarray(W1, dtype=np.float32).astype(bf16)
        per_in["W2"] = np.asarray(W2, dtype=np.float32).astype(bf16)
        per_in["b1"] = np.asarray(b1, dtype=np.float32).reshape(HID, 1)
        per_in["b2"] = np.asarray(b2, dtype=np.float32).reshape(OUT, 1)
        per_in["dinv_bc"] = np.broadcast_to(dinv_loc, (OUT, NPC)).copy()
        per_in["dinv_col"] = dinv_loc.reshape(NB, BW).T.copy()
        per_in["iota"] = np.broadcast_to(
            np.arange(BW, dtype=np.float32), (128, BW)).astype(bf16).copy()
        per_in["ident"] = np.eye(128, dtype=np.float32)
        inputs.append(per_in)
    return inputs, {"Tt": Tt, "Tt2": Tt2}


def _build_program(meta):
    Tt, Tt2 = meta["Tt"], meta["Tt2"]
    nc = bacc.Bacc("TRN2", target_bir_lowering=False, debug=False,
                   num_devices=N_CORES, num_swdge_queues=N_QUEUES,
                   dynamic_dma_scratch_size=DMA_SCRATCH)

    nt = {h: int(Tt[:, h].sum()) for h in (0, 1)}
    nt2 = {h: int(Tt2[:, h].sum()) for h in (0, 1)}

    msg_d = {h: nc.dram_tensor(f"msg{h}", [128, nt[h], IN], BF,
                               kind="ExternalInput") for h in (0, 1)}
    dstl_d = {h: nc.dram_tensor(f"dstl{h}", [128, nt[h]], BF,
                                kind="ExternalInput") for h in (0, 1)}
    dstl2_d = {h: nc.dram_tensor(f"dstl2_{h}", [128, nt2[h]], BF,
                                 kind="ExternalInput") for h in (0, 1)}
    idx_d = {h: nc.dram_tensor(f"idx{h}", [128, nt2[h] * 8],
                               mybir.dt.int16, kind="ExternalInput")
             for h in (0, 1)}
    W1_d = nc.dram_tensor("W1", [IN, HID], BF, kind="ExternalInput")
    W2_d = nc.dram_tensor("W2", [HID, OUT], BF, kind="ExternalInput")
    b1_d = nc.dram_tensor("b1", [HID, 1], F32, kind="ExternalInput")
    b2_d = nc.dram_tensor("b2", [OUT, 1], F32, kind="ExternalInput")
    dinvb_d = nc.dram_tensor("dinv_bc", [OUT, NPC], F32, kind="ExternalInput")
    dinvc_d = nc.dram_tensor("dinv_col", [BW, NB], F32, kind="ExternalInput")
    iota_d = nc.dram_tensor("iota", [128, BW], BF, kind="ExternalInput")
    id_d = nc.dram_tensor("ident", [128, 128], F32, kind="ExternalInput")
    out_d = nc.dram_tensor("out", [NPC, OUT], F32, kind="ExternalOutput")

    starts = np.zeros((NB, 2), dtype=np.int64)
    starts[1:, 0] = np.cumsum(Tt[:-1, 0])
    starts[1:, 1] = np.cumsum(Tt[:-1, 1])
    starts2 = np.zeros((NB, 2), dtype=np.int64)
    starts2[1:, 0] = np.cumsum(Tt2[:-1, 0])
    starts2[1:, 1] = np.cumsum(Tt2[:-1, 1])

    with tile.TileContext(nc) as tc:
        with (
            tc.tile_pool(name="consts", bufs=1) as consts,
            tc.tile_pool(name="msg", bufs=2) as msgp,
            tc.tile_pool(name="oh", bufs=2) as ohp,
            tc.tile_pool(name="aggs", bufs=2 * CHB) as aggsp,
            tc.tile_pool(name="act", bufs=2 * CHB) as actp,
            tc.tile_pool(name="outs", bufs=2 * CHB) as outsp,
            tc.tile_pool(name="agg_ps", bufs=4, space="PSUM") as agg_ps,
            tc.tile_pool(name="tr_ps", bufs=2, space="PSUM") as tr_ps,
            tc.tile_pool(name="tp_ps", bufs=2, space="PSUM") as tp_ps,
            tc.tile_pool(name="dram", bufs=1, space="DRAM") as dram,
        ):
            def load_const(name, dram_t, shape, dt):
                t = consts.tile(shape, dt, name=name, tag=name)
                nc.sync.dma_start(t[:], dram_t[:])
                return t

            # constants needed by layer 1 (the layer-2 idx/dstl loads are
            # issued after the collective below, filling its DMA-idle
            # window)
            dstl1_sb = {h: load_const(f"dstlsb{h}", dstl_d[h],
                                      [128, nt[h]], BF) for h in (0, 1)}
            W1_sb = load_const("w1", W1_d, [IN, HID], BF)
            W2_sb = load_const("w2", W2_d, [HID, OUT], BF)
            b1_sb = load_const("b1c", b1_d, [HID, 1], F32)
            b2_sb = load_const("b2c", b2_d, [OUT, 1], F32)
            dinvc_sb = load_const("dinvc", dinvc_d, [BW, NB], F32)
            iota_sb = load_const("iotac", iota_d, [128, BW], BF)
            idf_sb = load_const("idf", id_d, [128, 128], F32)
            idb_sb = consts.tile([128, 128], BF, tag="idb")
            nc.vector.tensor_copy(idb_sb[:], idf_sb[:])
            # feature-major copy of the local table shard (dinv[v]-scaled
            # relu(h1)@W2): the self-loop contribution to layer 2, added
            # densely in its tail instead of gathered per edge
            tloc = consts.tile([OUT, NPC], BF, tag="tloc")

            gq = [0]  # round-robin gather queue counter

            def onehot(dstl_sb, c0, tg, h):
                o_t = ohp.tile([128, tg, BW], BF, tag=f"oh{h}")
                iota_b = iota_sb[:].rearrange(
                    "p (o f) -> p o f", o=1).broadcast_to((128, tg, BW))
                dstl_b = dstl_sb[:, c0:c0 + tg].rearrange(
                    "p (t o) -> p t o", o=1).broadcast_to((128, tg, BW))
                nc.vector.tensor_tensor(
                    o_t[:], iota_b, dstl_b, mybir.AluOpType.is_equal)
                return o_t

            def gather(tblp, idx_sb, c0, tg, h):
                m_t = msgp.tile([128, tg, 2 * OUT], BF, tag=f"msg{h}")
                for g1 in range(0, tg, GSUB):
                    gn = min(GSUB, tg - g1)
                    nc.gpsimd.dma_gather(
                        out_ap=m_t[:, g1:g1 + gn, :],
                        in_ap=tblp,
                        idxs_ap=idx_sb[:, (c0 + g1) * 8:(c0 + g1 + gn) * 8],
                        num_idxs=gn * 128,
                        num_idxs_reg=gn * 128,
                        elem_size=2 * OUT,
                        single_packet=True,
                        queue_num=gq[0] % N_QUEUES,
                    )
                    gq[0] += 1
                return m_t

            def scatter_block(A, msg, oh, b, width, TtX, startsX):
                tot = int(TtX[b, 0] + TtX[b, 1])
                k = 0
                for h in (0, 1):
                    m_t, chunk0 = msg[h]
                    j0 = int(startsX[b, h]) - chunk0
                    for j in range(int(TtX[b, h])):
                        lhs = (m_t[:, j0 + j, :] if width == 128
                               else m_t[:, j0 + j, h * OUT:(h + 1) * OUT])
                        nc.tensor.matmul(
                            A[:], lhs, oh[h][:, j0 + j, :],
                            start=(k == 0), stop=(k == tot - 1))
                        k += 1

            # ---------------- layer 1 (streamed bf16 messages) ----------
            ag_in = dram.tile([NPC, OUT], BF, name="ag_in", tag="ag_in")
            ag_out = dram.tile([N, OUT], BF, addr_space="Shared",
                               name="ag_out", tag="ag_out")

            for g0 in range(0, NB, CHB):
                blocks = list(range(g0, min(g0 + CHB, NB)))
                msg = {}
                oh = {}
                for h in (0, 1):
                    c0 = int(starts[blocks[0], h])
                    tg = int(sum(Tt[b, h] for b in blocks))
                    m_t = msgp.tile([128, tg, IN], BF, tag=f"msg{h}")
                    # two HWDGE queues (SP + Activation) double the
                    # engine concurrency of the message stream
                    eng = nc.sync if h == 0 else nc.scalar
                    eng.dma_start(m_t[:], msg_d[h][:, c0:c0 + tg, :])
                    msg[h] = (m_t, c0)
                    oh[h] = onehot(dstl1_sb[h], c0, tg, h)
                for b in blocks:
                    A = agg_ps.tile([IN, BW], F32, tag="agg")
                    scatter_block(A, msg, oh, b, 128, Tt, starts)
                    aggs = aggsp.tile([128, BW], BF, tag="aggs")
                    nc.scalar.activation(
                        aggs[:], A[:], mybir.ActivationFunctionType.Copy)
                    P2 = tr_ps.tile([HID, BW], F32, tag="tr")
                    nc.tensor.matmul(P2[:], W1_sb[:], aggs[:],
                                     start=True, stop=True)
                    h1t = actp.tile([HID, BW], BF, tag="act")
                    nc.scalar.activation(
                        h1t[:], P2[:], mybir.ActivationFunctionType.Relu,
                        bias=b1_sb[:], scale=1.0)
                    P3 = tp_ps.tile([BW, OUT], F32, tag="tp")
                    nc.tensor.matmul(P3[:], h1t[:], W2_sb[:],
                                     start=True, stop=True)
                    t2 = outsp.tile([BW, OUT], BF, tag="t2")
                    nc.scalar.activation(
                        t2[:], P3[:], mybir.ActivationFunctionType.Copy,
                        bias=0.0, scale=dinvc_sb[:, b:b + 1])
                    nc.sync.dma_start(ag_in[b * BW:(b + 1) * BW, :], t2[:])
                    # feature-major copy for the layer-2 self-loop term
                    # (PE + Scalar have slack in layer 1; DVE does not)
                    P5 = tp_ps.tile([OUT, BW], BF, tag="tp")
                    nc.tensor.transpose(P5[:], t2[:], idb_sb[:BW, :BW])
                    nc.scalar.activation(
                        tloc[:, b * BW:(b + 1) * BW], P5[:],
                        mybir.ActivationFunctionType.Copy)

            # ---------------- AllGather ----------------
            nc.gpsimd.collective_compute(
                "AllGather",
                mybir.AluOpType.bypass,
                replica_groups=[list(range(N_CORES))],
                ins=[ag_in.opt()],
                outs=[ag_out.opt()],
            )

            # layer-2 constants (~6 MB): loaded during the collective on
            # both HWDGE queues (loading them mid-layer-1 was tried and
            # just slowed layer 1 by the same amount it saved here)
            idx_sb = {}
            for h in (0, 1):
                t = consts.tile([128, nt2[h] * 8], mybir.dt.int16,
                                name=f"idxsb{h}", tag=f"idxsb{h}")
                (nc.sync if h == 0 else nc.scalar).dma_start(
                    t[:], idx_d[h][:])
                idx_sb[h] = t
            dstl2_sb = {h: load_const(f"dstl2sb{h}", dstl2_d[h],
                                      [128, nt2[h]], BF) for h in (0, 1)}
            dinvb_sb = load_const("dinvb", dinvb_d, [OUT, NPC], F32)

            # ---------------- layer 2 (self loops folded via tloc) -----
            tblp = ag_out[:].rearrange("(m t) f -> m (t f)", t=2)
            for g0 in range(0, NB, CHB):
                blocks = list(range(g0, min(g0 + CHB, NB)))
                msg = {}
                oh = {}
                for h in (0, 1):
                    c0 = int(starts2[blocks[0], h])
                    tg = int(sum(Tt2[b, h] for b in blocks))
                    msg[h] = (gather(tblp, idx_sb[h], c0, tg, h), c0)
                    oh[h] = onehot(dstl2_sb[h], c0, tg, h)
                for b in blocks:
                    A = agg_ps.tile([OUT, BW], F32, tag="agg")
                    scatter_block(A, msg, oh, b, OUT, Tt2, starts2)
                    A2 = aggsp.tile([OUT, BW], F32, tag="aggs3")
                    nc.vector.tensor_tensor(
                        A2[:], A[:], tloc[:, b * BW:(b + 1) * BW],
                        mybir.AluOpType.add)
                    aggs = aggsp.tile([OUT, BW], F32, tag="aggs2")
                    nc.vector.tensor_tensor(
                        aggs[:], A2[:], dinvb_sb[:, b * BW:(b + 1) * BW],
                        mybir.AluOpType.mult)
                    ot = actp.tile([OUT, BW], BF, tag="act2")
                    b2_b = b2_sb[:].broadcast_to((OUT, BW))
                    nc.vector.tensor_tensor(
                        ot[:], aggs[:], b2_b, mybir.AluOpType.add)
                    P3 = tp_ps.tile([BW, OUT], BF, tag="tp")
                    nc.tensor.transpose(P3[:], ot[:], idb_sb[:OUT, :OUT])
                    t2 = outsp.tile([BW, OUT], F32, tag="t2f")
                    nc.scalar.activation(
                        t2[:], P3[:], mybir.ActivationFunctionType.Copy)
                    nc.sync.dma_start(
                        out_d[b * BW:(b + 1) * BW, :], t2[:])

    nc.compile()
    return nc


def kernel(x, edge_index, W1, b1, W2, b2):
    inputs, meta = _preprocess(x, edge_index, W1, b1, W2, b2)
    nc = _build_program(meta)
    res = run_bass_kernel_spmd(nc, inputs, core_ids=list(range(N_CORES)))
    out = np.concatenate(
        [res.results[m]["out"] for m in range(N_CORES)], axis=0)
    return out.astype(np.float32)


# revision 45
# speedup vs baseline: 1.2444x; 1.2444x over previous
"""2-layer GCN encoder on 8 TRN2 NeuronCores (Bass/Tile SPMD).

Strategy (per sharding hint): dst-node sharding, 6250 nodes/core.
- Host: compute degrees/norm, build per-core edge streams grouped by
  (dst block of 125 nodes, src parity), padded to 128-edge tiles with
  tile counts uniform across cores (one SPMD program).
- Layer 1: host-gathered, n_e-scaled bf16 messages are streamed on BOTH
  HWDGE queues (h=0 via SP, h=1 via Activation -- doubles DMA engine
  concurrency, 135 -> 115 us; fp8 messages were tried and pushed
  max-rel err to 0.033 > the 0.02 gate), segment-summed per dst block
  via one-hot matmul in PSUM, then W1 -> relu -> W2 fused tail; rows
  scaled by d^-1/2 and written to the bf16 inter-layer table shard.
- AllGather of the 6250x64 bf16 shards; the layer-2 gather index
  streams (~6 MB) are loaded concurrently with the collective (DMA is
  otherwise idle there).  A local/remote edge split that overlapped
  local-source aggregation with the collective was tried and REGRESSED
  (all phases here are HBM-bandwidth-bound, so overlapping two
  DMA-heavy phases just splits bandwidth and stretched the collective
  2.5x; it also added 13% more gather slots).  Deeper tile-pool
  buffering (bufs=3) and feature-major output writes were also tried
  and measured net-negative.
- Layer 2: dma_gather row-pairs from the AllGather'd table (int16 pair
  indices stay < 32768), one-hot scatter into PSUM, scale by
  d^-1/2[dst] (DVE), add b2, transpose, write output rows.
  Gather calls round-robin the 4 SWDGE queues.  single_packet=True
  calls are limited to GSUB=8 tiles (the per-engine packet is
  GSUB*128/16 * 256B and must stay <= PACKET_BYTES=16384; bigger calls
  hang the DMA engines).
"""
import numpy as np
import ml_dtypes

from concourse import bass, bacc, mybir, tile
from concourse.bass_utils import run_bass_kernel_spmd

N_CORES = 8
N = 50000
IN = 128
HID = 128
OUT = 64
NPC = N // N_CORES      # 6250 nodes per core
BW = 125                # dst block width
NB = NPC // BW          # 50 blocks per core
CHB = 5                 # blocks per processing chunk
N_QUEUES = 4            # SWDGE queues to round-robin dma_gather across
DMA_SCRATCH = 32768

# single_packet=False was A/B-tested and lost (fast 256B-packet drains
# but ~2x descriptor-generation cost starves them); keep True + GSUB=8.
GSUB = 8

BF = mybir.dt.bfloat16
F32 = mybir.dt.float32
bf16 = ml_dtypes.bfloat16


def _wrap_idx(idx):
    """dma_gather int16 index layout: [128, n/16]; index i at [i%16, i//16],
    replicated across the 8 gpsimd cores (16-partition groups)."""
    n = len(idx)
    assert n % 128 == 0
    base = np.asarray(idx, dtype=np.int16).reshape(n // 16, 16).T  # [16, n/16]
    return np.tile(base, (8, 1))


def _preprocess(x, edge_index, W1, b1, W2, b2):
    src = np.asarray(edge_index[0], dtype=np.int64)
    dst = np.asarray(edge_index[1], dtype=np.int64)
    loop = np.arange(N, dtype=np.int64)
    src = np.concatenate([src, loop])
    dst = np.concatenate([dst, loop])

    deg = np.bincount(dst, minlength=N).astype(np.float32)
    dinv = (1.0 / np.sqrt(deg)).astype(np.float32)  # deg >= 1 (self loops)

    x32 = np.asarray(x, dtype=np.float32)

    core = dst // NPC
    per_core = []
    cnts = np.zeros((N_CORES, NB, 2), dtype=np.int64)
    for m in range(N_CORES):
        sel = core == m
        s = src[sel]
        d = dst[sel] - m * NPC
        b = d // BW
        h = (s % 2).astype(np.int64)
        ne = dinv[s] * dinv[dst[sel]]
        per_core.append((s, b, h, ne, d % BW))
        cnts[m] = np.bincount(b * 2 + h, minlength=2 * NB).reshape(NB, 2)

    Tt = np.maximum(1, -(-cnts.max(axis=0) // 128))  # [NB, 2] tiles, >=1

    inputs = []
    for m in range(N_CORES):
        s, b, h, ne, l = per_core[m]
        per_in = {}
        for hh in (0, 1):
            mh = h == hh
            bh, lh, sh, neh = b[mh], l[mh], s[mh], ne[mh]
            # sort by src within each block bucket: ascending gather
            # addresses improve HBM locality
            order = np.lexsort((sh, bh))
            bh, lh, sh, neh = bh[order], lh[order], sh[order], neh[order]
            bounds = np.searchsorted(bh, np.arange(NB + 1))
            chunks_idx, chunks_dstl, chunks_msg = [], [], []
            for bb in range(NB):
                lo, hi = bounds[bb], bounds[bb + 1]
                npad = Tt[bb, hh] * 128 - (hi - lo)
                assert npad >= 0
                chunks_idx.append(sh[lo:hi] // 2)
                chunks_idx.append(np.zeros(npad, dtype=np.int64))
                chunks_dstl.append(lh[lo:hi])
                chunks_dstl.append(np.full(npad, 126, dtype=np.int64))
                chunks_msg.append(x32[sh[lo:hi]] * neh[lo:hi, None])
                chunks_msg.append(np.zeros((npad, IN), dtype=np.float32))
            dstl = np.concatenate(chunks_dstl)
            ntile = len(dstl) // 128
            per_in[f"idx{hh}"] = _wrap_idx(np.concatenate(chunks_idx))
            per_in[f"dstl{hh}"] = dstl.reshape(ntile, 128).T.astype(bf16).copy()
            msg = np.concatenate(chunks_msg).astype(bf16)
            per_in[f"msg{hh}"] = np.ascontiguousarray(
                msg.reshape(ntile, 128, IN).transpose(1, 0, 2))

        mloc = m * NPC
        dinv_loc = dinv[mloc:mloc + NPC]
        per_in["W1"] = np.asarray(W1, dtype=np.float32).astype(bf16)
        per_in["W2"] = np.asarray(W2, dtype=np.float32).astype(bf16)
        per_in["b1"] = np.asarray(b1, dtype=np.float32).reshape(HID, 1)
        per_in["b2"] = np.asarray(b2, dtype=np.float32).reshape(OUT, 1)
        per_in["dinv_bc"] = np.broadcast_to(dinv_loc, (OUT, NPC)).copy()
        per_in["dinv_col"] = dinv_loc.reshape(NB, BW).T.copy()
        per_in["iota"] = np.broadcast_to(
            np.arange(BW, dtype=np.float32), (128, BW)).astype(bf16).copy()
        per_in["ident"] = np.eye(128, dtype=np.float32)
        inputs.append(per_in)
    return inputs, {"Tt": Tt}


def _build_program(meta):
    Tt = meta["Tt"]
    nc = bacc.Bacc("TRN2", target_bir_lowering=False, debug=False,
                   num_devices=N_CORES, num_swdge_queues=N_QUEUES,
                   dynamic_dma_scratch_size=DMA_SCRATCH)

    nt = {h: int(Tt[:, h].sum()) for h in (0, 1)}

    msg_d = {h: nc.dram_tensor(f"msg{h}", [128, nt[h], IN], BF,
                               kind="ExternalInput") for h in (0, 1)}
    dstl_d = {h: nc.dram_tensor(f"dstl{h}", [128, nt[h]], BF,
                                kind="ExternalInput") for h in (0, 1)}
    idx_d = {h: nc.dram_tensor(f"idx{h}", [128, nt[h] * 8],
                               mybir.dt.int16, kind="ExternalInput")
             for h in (0, 1)}
    W1_d = nc.dram_tensor("W1", [IN, HID], BF, kind="ExternalInput")
    W2_d = nc.dram_tensor("W2", [HID, OUT], BF, kind="ExternalInput")
    b1_d = nc.dram_tensor("b1", [HID, 1], F32, kind="ExternalInput")
    b2_d = nc.dram_tensor("b2", [OUT, 1], F32, kind="ExternalInput")
    dinvb_d = nc.dram_tensor("dinv_bc", [OUT, NPC], F32, kind="ExternalInput")
    dinvc_d = nc.dram_tensor("dinv_col", [BW, NB], F32, kind="ExternalInput")
    iota_d = nc.dram_tensor("iota", [128, BW], BF, kind="ExternalInput")
    id_d = nc.dram_tensor("ident", [128, 128], F32, kind="ExternalInput")
    out_d = nc.dram_tensor("out", [NPC, OUT], F32, kind="ExternalOutput")

    starts = np.zeros((NB, 2), dtype=np.int64)
    starts[1:, 0] = np.cumsum(Tt[:-1, 0])
    starts[1:, 1] = np.cumsum(Tt[:-1, 1])

    with tile.TileContext(nc) as tc:
        with (
            tc.tile_pool(name="consts", bufs=1) as consts,
            tc.tile_pool(name="msg", bufs=2) as msgp,
            tc.tile_pool(name="oh", bufs=2) as ohp,
            tc.tile_pool(name="aggs", bufs=2 * CHB) as aggsp,
            tc.tile_pool(name="act", bufs=2 * CHB) as actp,
            tc.tile_pool(name="outs", bufs=2 * CHB) as outsp,
            tc.tile_pool(name="agg_ps", bufs=4, space="PSUM") as agg_ps,
            tc.tile_pool(name="tr_ps", bufs=2, space="PSUM") as tr_ps,
            tc.tile_pool(name="tp_ps", bufs=2, space="PSUM") as tp_ps,
            tc.tile_pool(name="dram", bufs=1, space="DRAM") as dram,
        ):
            def load_const(name, dram_t, shape, dt):
                t = consts.tile(shape, dt, name=name, tag=name)
                nc.sync.dma_start(t[:], dram_t[:])
                return t

            # constants needed by layer 1 (the layer-2 idx/dstl loads are
            # issued after the collective below, filling its DMA-idle
            # window)
            dstl1_sb = {h: load_const(f"dstlsb{h}", dstl_d[h],
                                      [128, nt[h]], BF) for h in (0, 1)}
            W1_sb = load_const("w1", W1_d, [IN, HID], BF)
            W2_sb = load_const("w2", W2_d, [HID, OUT], BF)
            b1_sb = load_const("b1c", b1_d, [HID, 1], F32)
            b2_sb = load_const("b2c", b2_d, [OUT, 1], F32)
            dinvc_sb = load_const("dinvc", dinvc_d, [BW, NB], F32)
            iota_sb = load_const("iotac", iota_d, [128, BW], BF)
            idf_sb = load_const("idf", id_d, [128, 128], F32)
            idb_sb = consts.tile([128, 128], BF, tag="idb")
            nc.vector.tensor_copy(idb_sb[:], idf_sb[:])

            gq = [0]  # round-robin gather queue counter

            def onehot(dstl_sb, c0, tg, h):
                o_t = ohp.tile([128, tg, BW], BF, tag=f"oh{h}")
                iota_b = iota_sb[:].rearrange(
                    "p (o f) -> p o f", o=1).broadcast_to((128, tg, BW))
                dstl_b = dstl_sb[:, c0:c0 + tg].rearrange(
                    "p (t o) -> p t o", o=1).broadcast_to((128, tg, BW))
                nc.vector.tensor_tensor(
                    o_t[:], iota_b, dstl_b, mybir.AluOpType.is_equal)
                return o_t

            def gather(tblp, idx_sb, c0, tg, h):
                m_t = msgp.tile([128, tg, 2 * OUT], BF, tag=f"msg{h}")
                for g1 in range(0, tg, GSUB):
                    gn = min(GSUB, tg - g1)
                    nc.gpsimd.dma_gather(
                        out_ap=m_t[:, g1:g1 + gn, :],
                        in_ap=tblp,
                        idxs_ap=idx_sb[:, (c0 + g1) * 8:(c0 + g1 + gn) * 8],
                        num_idxs=gn * 128,
                        num_idxs_reg=gn * 128,
                        elem_size=2 * OUT,
                        single_packet=True,
                        queue_num=gq[0] % N_QUEUES,
                    )
                    gq[0] += 1
                return m_t

            def scatter_block(A, msg, oh, b, width):
                tot = int(Tt[b, 0] + Tt[b, 1])
                k = 0
                for h in (0, 1):
                    m_t, chunk0 = msg[h]
                    j0 = int(starts[b, h]) - chunk0
                    for j in range(int(Tt[b, h])):
                        lhs = (m_t[:, j0 + j, :] if width == 128
                               else m_t[:, j0 + j, h * OUT:(h + 1) * OUT])
                        nc.tensor.matmul(
                            A[:], lhs, oh[h][:, j0 + j, :],
                            start=(k == 0), stop=(k == tot - 1))
                        k += 1

            # ---------------- layer 1 (streamed bf16 messages) ----------
            ag_in = dram.tile([NPC, OUT], BF, name="ag_in", tag="ag_in")
            ag_out = dram.tile([N, OUT], BF, addr_space="Shared",
                               name="ag_out", tag="ag_out")

            for g0 in range(0, NB, CHB):
                blocks = list(range(g0, min(g0 + CHB, NB)))
                msg = {}
                oh = {}
                for h in (0, 1):
                    c0 = int(starts[blocks[0], h])
                    tg = int(sum(Tt[b, h] for b in blocks))
                    m_t = msgp.tile([128, tg, IN], BF, tag=f"msg{h}")
                    # two HWDGE queues (SP + Activation) double the
                    # engine concurrency of the message stream
                    eng = nc.sync if h == 0 else nc.scalar
                    eng.dma_start(m_t[:], msg_d[h][:, c0:c0 + tg, :])
                    msg[h] = (m_t, c0)
                    oh[h] = onehot(dstl1_sb[h], c0, tg, h)
                for b in blocks:
                    A = agg_ps.tile([IN, BW], F32, tag="agg")
                    scatter_block(A, msg, oh, b, 128)
                    aggs = aggsp.tile([128, BW], BF, tag="aggs")
                    nc.scalar.activation(
                        aggs[:], A[:], mybir.ActivationFunctionType.Copy)
                    P2 = tr_ps.tile([HID, BW], F32, tag="tr")
                    nc.tensor.matmul(P2[:], W1_sb[:], aggs[:],
                                     start=True, stop=True)
                    h1t = actp.tile([HID, BW], BF, tag="act")
                    nc.scalar.activation(
                        h1t[:], P2[:], mybir.ActivationFunctionType.Relu,
                        bias=b1_sb[:], scale=1.0)
                    P3 = tp_ps.tile([BW, OUT], F32, tag="tp")
                    nc.tensor.matmul(P3[:], h1t[:], W2_sb[:],
                                     start=True, stop=True)
                    t2 = outsp.tile([BW, OUT], BF, tag="t2")
                    nc.scalar.activation(
                        t2[:], P3[:], mybir.ActivationFunctionType.Copy,
                        bias=0.0, scale=dinvc_sb[:, b:b + 1])
                    nc.sync.dma_start(ag_in[b * BW:(b + 1) * BW, :], t2[:])

            # ---------------- AllGather ----------------
            nc.gpsimd.collective_compute(
                "AllGather",
                mybir.AluOpType.bypass,
                replica_groups=[list(range(N_CORES))],
                ins=[ag_in.opt()],
                outs=[ag_out.opt()],
            )

            # layer-2 constants (~6 MB): loaded during the collective on
            # both HWDGE queues (loading them mid-layer-1 was tried and
            # just slowed layer 1 by the same amount it saved here)
            idx_sb = {}
            for h in (0, 1):
                t = consts.tile([128, nt[h] * 8], mybir.dt.int16,
                                name=f"idxsb{h}", tag=f"idxsb{h}")
                (nc.sync if h == 0 else nc.scalar).dma_start(
                    t[:], idx_d[h][:])
                idx_sb[h] = t
            dinvb_sb = load_const("dinvb", dinvb_d, [OUT, NPC], F32)

            # ---------------- layer 2 ----------------
            tblp = ag_out[:].rearrange("(m t) f -> m (t f)", t=2)
            for g0 in range(0, NB, CHB):
                blocks = list(range(g0, min(g0 + CHB, NB)))
                msg = {}
                oh = {}
                for h in (0, 1):
                    c0 = int(starts[blocks[0], h])
                    tg = int(sum(Tt[b, h] for b in blocks))
                    msg[h] = (gather(tblp, idx_sb[h], c0, tg, h), c0)
                    oh[h] = onehot(dstl1_sb[h], c0, tg, h)
                for b in blocks:
                    A = agg_ps.tile([OUT, BW], F32, tag="agg")
                    scatter_block(A, msg, oh, b, OUT)
                    aggs = aggsp.tile([OUT, BW], F32, tag="aggs2")
                    nc.vector.tensor_tensor(
                        aggs[:], A[:], dinvb_sb[:, b * BW:(b + 1) * BW],
                        mybir.AluOpType.mult)
                    ot = actp.tile([OUT, BW], BF, tag="act2")
                    b2_b = b2_sb[:].broadcast_to((OUT, BW))
                    nc.vector.tensor_tensor(
                        ot[:], aggs[:], b2_b, mybir.AluOpType.add)
                    P3 = tp_ps.tile([BW, OUT], BF, tag="tp")
                    nc.tensor.transpose(P3[:], ot[:], idb_sb[:OUT, :OUT])
                    t2 = outsp.tile([BW, OUT], F32, tag="t2f")
                    nc.scalar.activation(
                        t2[:], P3[:], mybir.ActivationFunctionType.Copy)
                    nc.sync.dma_start(
                        out_d[b * BW:(b + 1) * BW, :], t2[:])

    nc.compile()
    return nc


def kernel(x, edge_index, W1, b1, W2, b2):
    inputs, meta = _preprocess(x, edge_index, W1, b1, W2, b2)
    nc = _build_program(meta)
    res = run_bass_kernel_spmd(nc, inputs, core_ids=list(range(N_CORES)))
    out = np.concatenate(
        [res.results[m]["out"] for m in range(N_CORES)], axis=0)
    return out.astype(np.float32)
